# revision 25
# baseline (speedup 1.0000x reference)
import os
import sys

sys.path.insert(0, "/opt/trn_rl_repo")

import numpy as np

# Problem constants (hardcoded per harness contract)
B = 64          # full batch
NC_CORES = 8
BPC = 8         # batches per core
N = 1024
D = 768
NS = 16         # n_slots
KT = 8          # n-tiles of 128
DT = 6          # d-tiles of 128
DH = D // 2     # 384, nibble-pack pairing half

_CACHE = {}


def _build_nc(debug=False):
    import concourse.bacc as bacc
    import concourse.tile as tile
    import concourse.mybir as mybir
    from concourse.bass import IndirectOffsetOnAxis

    fp32 = mybir.dt.float32
    fp16 = mybir.dt.float16
    bf16 = mybir.dt.bfloat16
    i16 = mybir.dt.int16
    u8 = mybir.dt.uint8
    i32 = mybir.dt.int32
    u32 = mybir.dt.uint32
    Alu = mybir.AluOpType
    Act = mybir.ActivationFunctionType

    nc = bacc.Bacc(
        "TRN2",
        target_bir_lowering=False,
        debug=False,
        enable_asserts=False,
        num_devices=NC_CORES,
    )

    # 20-bit quantized features:
    #   q1 int16 = rint(f/s1);  p uint8 = packed int4 residual pairs:
    #       byte = (q4[d]+8) + 16*(q4[d+384]+8),  q4 = rint((f/s1 - q1)*14)
    # Reconstruction: f' = q1 + q4/14   (in units of s1; normalize cancels
    # the scale, s1 is folded back via aux[:,0] into the final slot scaling).
    # Kept as two separate inputs: concatenating them inside the quantize jit
    # makes XLA recompute the encode per concat consumer (2x host cost).
    q1_dr = nc.dram_tensor("q1", [BPC, N, D], i16, kind="ExternalInput").ap()
    p_dr = nc.dram_tensor("p4", [BPC, N, DH], u8, kind="ExternalInput").ap()
    # aux: [:,0]=s1, [:,1]=rowbase (first BPC rows), [:,2:130]=identity
    aux_dr = nc.dram_tensor("aux", [128, 130], fp32, kind="ExternalInput").ap()
    out_dr = nc.dram_tensor("slots", [BPC, NS, D], fp16, kind="ExternalOutput").ap()
    g_dr = nc.dram_tensor("g_scratch", [BPC * N, N], fp32, kind="Internal").ap()

    with tile.TileContext(nc) as tc:
        with (
            tc.tile_pool(name="main", bufs=1) as mp,
            tc.tile_pool(name="fbuf", bufs=2) as fbp,
            tc.tile_pool(name="qbuf", bufs=2) as qbp,
            tc.tile_pool(name="scr", bufs=1) as scrp,
            tc.tile_pool(name="fnt", bufs=1) as ftp,
            tc.tile_pool(name="gst", bufs=4) as gsp,
            tc.tile_pool(name="small", bufs=2) as smp,
            tc.tile_pool(name="psA", bufs=2, space="PSUM") as ppA,
            tc.tile_pool(name="psB", bufs=2, space="PSUM") as ppB,
        ):
            aux = mp.tile([128, 130], fp32)
            nc.sync.dma_start(aux, aux_dr)
            ident = aux[:, 2:130]
            rowb = aux[0:BPC, 1:2]
            scl = aux[:, 0:1]

            # persistent across phases
            sal_loop = mp.tile([BPC, N], fp32)             # saliency, loop layout
            wT = mp.tile([128, KT, BPC, NS], fp32)         # slot weights, lhsT layout
            wsum = mp.tile([BPC, NS], fp32)

            def load_f(b):
                """DMA quantized batch b and reconstruct fp32 f' (units of s1)."""
                q1_sb = qbp.tile([128, KT, D], i16, tag="q1")
                nc.sync.dma_start(
                    q1_sb, q1_dr[b].rearrange("(kt p) d -> p kt d", p=128)
                )
                p_sb = qbp.tile([128, KT, DH], u8, tag="p4")
                nc.sync.dma_start(
                    p_sb, p_dr[b].rearrange("(kt p) d -> p kt d", p=128)
                )
                n0 = scrp.tile([128, KT, DH], u8, tag="n0")
                n1 = scrp.tile([128, KT, DH], u8, tag="n1")
                nc.vector.tensor_scalar(n0, p_sb, 15, None, op0=Alu.bitwise_and)
                nc.vector.tensor_scalar(
                    n1, p_sb, 4, None, op0=Alu.logical_shift_right
                )
                f_sb = fbp.tile([128, KT, D], fp32, tag="f")
                t4 = scrp.tile([128, KT, DH], fp32, tag="t4")
                # (n - 8) / 14, with int->fp32 cast, on the scalar engine
                nc.scalar.activation(
                    t4, n0, Act.Copy, scale=1.0 / 14.0, bias=-8.0 / 14.0,
                )
                nc.vector.tensor_add(
                    f_sb[:, :, 0:DH], t4, q1_sb[:, :, 0:DH]
                )
                t4b = scrp.tile([128, KT, DH], fp32, tag="t4")
                nc.scalar.activation(
                    t4b, n1, Act.Copy, scale=1.0 / 14.0, bias=-8.0 / 14.0,
                )
                nc.vector.tensor_add(
                    f_sb[:, :, DH:D], t4b, q1_sb[:, :, DH:D]
                )
                return f_sb

            # ---------------- Phase A: per-batch normalize + Gram ----------
            for b in range(BPC):
                f_sb = load_f(b)
                sal2 = smp.tile([128, KT], fp32, tag="sal2")
                sq_scr = smp.tile([128, D], fp32, tag="sqscr")
                for kt in range(KT):
                    nc.scalar.activation(
                        sq_scr, f_sb[:, kt], Act.Square,
                        accum_out=sal2[:, kt:kt + 1],
                    )
                salb = smp.tile([128, KT], fp32, tag="salb")
                nc.scalar.activation(salb, sal2, Act.Sqrt)
                invb = smp.tile([128, KT], fp32, tag="invb")
                nc.vector.reciprocal(invb, salb)

                # saliency into loop layout [1, N] via PE transpose
                salT_ps = ppB.tile([KT, 128], fp32, tag="tps")
                nc.tensor.transpose(salT_ps, salb, ident)
                salT = smp.tile([KT, 128], fp32, tag="salT")
                nc.scalar.copy(salT, salT_ps)
                nc.sync.dma_start(sal_loop[b:b + 1, :], salT[:, :])

                # scale f in place -> fn (unit rows)
                for kt in range(KT):
                    nc.vector.tensor_scalar(
                        f_sb[:, kt], f_sb[:, kt], invb[:, kt:kt + 1], None,
                        op0=Alu.mult,
                    )

                # transpose fn -> fnT [128(d), DT, N]
                fnT = ftp.tile([128, DT, N], fp32, tag="fnT")
                for kt in range(KT):
                    for dt in range(DT):
                        tp = ppB.tile([128, 128], fp32, tag="tps")
                        nc.tensor.transpose(
                            tp, f_sb[:, kt, dt * 128:(dt + 1) * 128], ident
                        )
                        if (kt + dt) % 2 == 0:
                            nc.scalar.copy(
                                fnT[:, dt, kt * 128:(kt + 1) * 128], tp
                            )
                        else:
                            nc.vector.tensor_copy(
                                fnT[:, dt, kt * 128:(kt + 1) * 128], tp
                            )

                # G = fnT.T @ fnT  (normalized Gram), row tiles -> DRAM
                for i in range(KT):
                    gps = ppA.tile([128, N], fp32, tag="gps")
                    for h in range(2):
                        for dt in range(DT):
                            nc.tensor.matmul(
                                gps[:, h * 512:(h + 1) * 512],
                                fnT[:, dt, i * 128:(i + 1) * 128],
                                fnT[:, dt, h * 512:(h + 1) * 512],
                                start=(dt == 0),
                                stop=(dt == DT - 1),
                            )
                    gstage = gsp.tile([128, N], fp32, tag="gstage")
                    nc.vector.tensor_copy(gstage[:, :512], gps[:, :512])
                    nc.scalar.copy(gstage[:, 512:], gps[:, 512:])
                    nc.sync.dma_start(
                        g_dr[b * N + i * 128: b * N + (i + 1) * 128, :], gstage
                    )

            # make sure all Gram writes to DRAM are visible before gathers
            tc.strict_bb_all_engine_barrier()

            # ---------------- Phase B: 16-step greedy loop -----------------
            mask = mp.tile([BPC, N], fp32)
            nc.vector.memset(mask, 1.0)
            msal = mp.tile([BPC, N], fp32)
            sim = mp.tile([BPC, N], fp32)
            mx8 = mp.tile([BPC, 8], fp32)
            idx8 = mp.tile([BPC, 8], u32)
            idxf = mp.tile([BPC, 1], fp32)
            rowidx = mp.tile([BPC, 1], i32)
            w1 = mp.tile([BPC, N], fp32)
            gate = mp.tile([BPC, N], fp32)
            aggw = mp.tile([BPC, N], fp32)
            aggw_bf = mp.tile([BPC, N], bf16)
            clipv = mp.tile([BPC, N], fp32)

            sim2 = mp.tile([BPC, N], fp32)
            w1b = mp.tile([BPC, N], fp32)
            sims = [sim, sim2]
            w1s = [w1, w1b]

            def emit_deferred(t):
                # off-critical aggregation work for step t (fills gather wait)
                s = sims[t % 2]
                w = w1s[t % 2]
                nc.vector.tensor_scalar(
                    gate, s, 0.5, None, op0=Alu.is_gt
                )
                nc.vector.tensor_mul(aggw, w, gate)
                nc.scalar.activation(
                    aggw_bf, aggw, Act.Copy,
                    accum_out=wsum[:, t:t + 1],
                )
                for kt in range(KT):
                    tp2 = ppB.tile([128, 128], fp32, tag="tps")
                    nc.tensor.transpose(
                        tp2[:, :BPC],
                        aggw[:, kt * 128:(kt + 1) * 128],
                        ident[:BPC, :BPC],
                    )
                    nc.scalar.copy(wT[:, kt, :, t], tp2[:, :BPC])

            for t in range(NS):
                s = sims[t % 2]
                nc.vector.tensor_mul(msal, sal_loop, mask)
                nc.vector.max(out=mx8, in_=msal)
                nc.vector.max_index(out=idx8, in_max=mx8, in_values=msal)
                nc.vector.tensor_copy(idxf, idx8[:, 0:1])
                nc.vector.tensor_scalar(
                    rowidx, idxf, rowb, None, op0=Alu.add
                )
                nc.gpsimd.indirect_dma_start(
                    out=s,
                    out_offset=None,
                    in_=g_dr,
                    in_offset=IndirectOffsetOnAxis(ap=rowidx, axis=0),
                )
                if t > 0:
                    emit_deferred(t - 1)
                # critical tail: uses gathered sim
                nc.vector.tensor_mul(w1s[t % 2], s, mask)
                nc.vector.tensor_scalar(
                    clipv, s, 0.0, 1.0, op0=Alu.max, op1=Alu.min
                )
                nc.vector.tensor_scalar(
                    clipv, clipv, -1.0, 1.0, op0=Alu.mult, op1=Alu.add
                )
                nc.vector.tensor_mul(mask, mask, clipv)
            emit_deferred(NS - 1)

            # ---------------- Phase C: slot matmuls ------------------------
            nc.vector.tensor_scalar(wsum, wsum, 1e-8, None, op0=Alu.add)
            recip = mp.tile([BPC, NS], fp32)
            nc.vector.reciprocal(recip, wsum)
            rT_ps = ppB.tile([128, 128], fp32, tag="tps")
            nc.tensor.transpose(rT_ps[:NS, :BPC], recip, ident[:BPC, :BPC])
            recipT = mp.tile([NS, BPC], fp32)
            # fold the quantization scale s1 back in: slot = sp * recip * s1
            nc.vector.tensor_scalar(
                recipT, rT_ps[:NS, :BPC], scl[:NS, 0:1], None, op0=Alu.mult
            )

            for b in range(BPC):
                f_c = load_f(b)
                sp = ppA.tile([NS, D], fp32, tag="gps")
                for h, (h0, h1) in enumerate([(0, 512), (512, D)]):
                    for kt in range(KT):
                        nc.tensor.matmul(
                            sp[:, h0:h1],
                            wT[:, kt, b, :],
                            f_c[:, kt, h0:h1],
                            start=(kt == 0),
                            stop=(kt == KT - 1),
                        )
                slot_sb = gsp.tile([NS, D], fp16, tag="slot")
                nc.scalar.activation(
                    slot_sb, sp, Act.Copy, scale=recipT[:, b:b + 1]
                )
                nc.sync.dma_start(out_dr[b], slot_sb)

    nc.compile()
    return nc


def _get_nc(debug=False):
    key = ("nc", debug)
    if key not in _CACHE:
        _CACHE[key] = _build_nc(debug)
    return _CACHE[key]


def _enable_jax_compile_cache():
    # run_bass_via_pjrt builds a fresh jax.jit closure per call; the
    # persistent cache turns each call's XLA recompile into a disk hit.
    if "jaxcache" in _CACHE:
        return
    _CACHE["jaxcache"] = True
    try:
        import tempfile
        import jax
        cache_dir = os.path.join(tempfile.gettempdir(), "kernel_jax_cache")
        os.makedirs(cache_dir, exist_ok=True)
        jax.config.update("jax_compilation_cache_dir", cache_dir)
        jax.config.update("jax_persistent_cache_min_compile_time_secs", 0.0)
        jax.config.update("jax_persistent_cache_min_entry_size_bytes", 0)
    except Exception:
        pass


def _quantize(features):
    """Host-side 20-bit encoding (int16 + packed int4) via jax-cpu jit."""
    import jax
    import jax.numpy as jnp

    cpu = jax.devices("cpu")[0]

    if "quant" not in _CACHE:
        def _q(f):
            # decimated max + 1.10 headroom; the clips below keep any
            # tail elements sound even if the subsample misses the true max
            decmax = jnp.max(jnp.abs(f[:, :, ::8]))
            s1 = decmax * np.float32(1.10) / np.float32(32767.0)
            inv_s1 = np.float32(1.0) / s1
            x = f * inv_s1
            q1f = jnp.clip(jnp.rint(x), -32767.0, 32767.0)
            q1 = q1f.astype(jnp.int16)
            r14 = jnp.clip((x - q1f) * 14.0, -7.0, 7.0)
            q4 = (jnp.rint(r14) + 8.0).astype(jnp.uint8)
            packed = q4[..., :DH] + q4[..., DH:] * jnp.uint8(16)
            return q1, packed, s1
        _CACHE["quant"] = jax.jit(_q, backend="cpu")

    f_np = np.ascontiguousarray(features, np.float32)
    try:
        with jax.default_device(cpu):
            f = jnp.from_dlpack(f_np)       # zero-copy np -> jax cpu
        assert f.dtype == jnp.float32
    except Exception:
        f = jax.device_put(f_np, cpu)
    q1, p4, s1 = _CACHE["quant"](f)
    try:
        q1 = np.from_dlpack(q1)
        p4 = np.from_dlpack(p4)
    except Exception:
        q1 = np.asarray(q1)
        p4 = np.asarray(p4)
    return q1, p4, np.float32(s1)


def kernel(features, batch_size=None, **_kw):
    from concourse import bass_utils

    _enable_jax_compile_cache()
    nc = _get_nc()
    q1, p4, s1 = _quantize(features)
    aux = np.zeros((128, 130), dtype=np.float32)
    aux[:, 0] = s1
    aux[:BPC, 1] = np.arange(BPC, dtype=np.float32) * N
    aux[:, 2:130] = np.eye(128, dtype=np.float32)
    in_maps = [
        {"q1": q1[i * BPC:(i + 1) * BPC], "p4": p4[i * BPC:(i + 1) * BPC],
         "aux": aux}
        for i in range(NC_CORES)
    ]
    res = bass_utils.run_bass_kernel_spmd(
        nc, in_maps, core_ids=list(range(NC_CORES))
    )
    outs = [np.asarray(res.results[i]["slots"]) for i in range(NC_CORES)]
    return np.concatenate(outs, axis=0).astype(np.float32)


# revision 26
# speedup vs baseline: 1.0216x; 1.0216x over previous
import os
import sys

sys.path.insert(0, "/opt/trn_rl_repo")

import numpy as np

# Problem constants (hardcoded per harness contract)
B = 64          # full batch
NC_CORES = 8
BPC = 8         # batches per core
N = 1024
D = 768
NS = 16         # n_slots
KT = 8          # n-tiles of 128
DT = 6          # d-tiles of 128
DH = D // 2     # 384, nibble-pack pairing half

_CACHE = {}


def _build_nc(debug=False):
    import concourse.bacc as bacc
    import concourse.tile as tile
    import concourse.mybir as mybir
    from concourse.bass import IndirectOffsetOnAxis

    fp32 = mybir.dt.float32
    fp16 = mybir.dt.float16
    bf16 = mybir.dt.bfloat16
    i16 = mybir.dt.int16
    u8 = mybir.dt.uint8
    i32 = mybir.dt.int32
    u32 = mybir.dt.uint32
    Alu = mybir.AluOpType
    Act = mybir.ActivationFunctionType

    nc = bacc.Bacc(
        "TRN2",
        target_bir_lowering=False,
        debug=False,
        enable_asserts=False,
        num_devices=NC_CORES,
    )

    # 20-bit quantized features:
    #   q1 int16 = rint(f/s1);  p uint8 = packed int4 residual pairs:
    #       byte = (q4[d]+8) + 16*(q4[d+384]+8),  q4 = rint((f/s1 - q1)*14)
    # Reconstruction: f' = q1 + q4/14   (in units of s1; normalize cancels
    # the scale, s1 is folded back via aux[:,0] into the final slot scaling).
    # Kept as two separate inputs: concatenating them inside the quantize jit
    # makes XLA recompute the encode per concat consumer (2x host cost).
    q1_dr = nc.dram_tensor("q1", [BPC, N, D], i16, kind="ExternalInput").ap()
    p_dr = nc.dram_tensor("p4", [BPC, N, DH], u8, kind="ExternalInput").ap()
    # aux: [:,0]=s1, [:,1]=rowbase (first BPC rows), [:,2:130]=identity
    aux_dr = nc.dram_tensor("aux", [128, 130], fp32, kind="ExternalInput").ap()
    out_dr = nc.dram_tensor("slots", [BPC, NS, D], fp16, kind="ExternalOutput").ap()
    g_dr = nc.dram_tensor("g_scratch", [BPC * N, N], fp32, kind="Internal").ap()

    with tile.TileContext(nc) as tc:
        with (
            tc.tile_pool(name="main", bufs=1) as mp,
            tc.tile_pool(name="fbuf", bufs=2) as fbp,
            tc.tile_pool(name="qbuf", bufs=2) as qbp,
            tc.tile_pool(name="scr", bufs=1) as scrp,
            tc.tile_pool(name="fnt", bufs=1) as ftp,
            tc.tile_pool(name="gst", bufs=4) as gsp,
            tc.tile_pool(name="small", bufs=2) as smp,
            tc.tile_pool(name="psA", bufs=2, space="PSUM") as ppA,
            tc.tile_pool(name="psB", bufs=2, space="PSUM") as ppB,
        ):
            aux = mp.tile([128, 130], fp32)
            nc.sync.dma_start(aux, aux_dr)
            ident = aux[:, 2:130]
            rowb = aux[0:BPC, 1:2]
            scl = aux[:, 0:1]

            # persistent across phases
            sal_loop = mp.tile([BPC, N], fp32)             # saliency, loop layout
            wT = mp.tile([128, KT, BPC, NS], fp32)         # slot weights, lhsT layout
            wsum = mp.tile([BPC, NS], fp32)

            def load_f(b):
                """DMA quantized batch b and reconstruct fp32 f' (units of s1)."""
                q1_sb = qbp.tile([128, KT, D], i16, tag="q1")
                nc.sync.dma_start(
                    q1_sb, q1_dr[b].rearrange("(kt p) d -> p kt d", p=128)
                )
                p_sb = qbp.tile([128, KT, DH], u8, tag="p4")
                nc.sync.dma_start(
                    p_sb, p_dr[b].rearrange("(kt p) d -> p kt d", p=128)
                )
                n0 = scrp.tile([128, KT, DH], u8, tag="n0")
                n1 = scrp.tile([128, KT, DH], u8, tag="n1")
                nc.vector.tensor_scalar(n0, p_sb, 15, None, op0=Alu.bitwise_and)
                nc.vector.tensor_scalar(
                    n1, p_sb, 4, None, op0=Alu.logical_shift_right
                )
                f_sb = fbp.tile([128, KT, D], fp32, tag="f")
                t4 = scrp.tile([128, KT, DH], fp32, tag="t4")
                # (n - 8) / 14, with int->fp32 cast, on the scalar engine
                nc.scalar.activation(
                    t4, n0, Act.Copy, scale=1.0 / 14.0, bias=-8.0 / 14.0,
                )
                nc.vector.tensor_add(
                    f_sb[:, :, 0:DH], t4, q1_sb[:, :, 0:DH]
                )
                t4b = scrp.tile([128, KT, DH], fp32, tag="t4")
                nc.scalar.activation(
                    t4b, n1, Act.Copy, scale=1.0 / 14.0, bias=-8.0 / 14.0,
                )
                nc.vector.tensor_add(
                    f_sb[:, :, DH:D], t4b, q1_sb[:, :, DH:D]
                )
                return f_sb

            # ---------------- Phase A: per-batch normalize + Gram ----------
            for b in range(BPC):
                f_sb = load_f(b)
                sal2 = smp.tile([128, KT], fp32, tag="sal2")
                sq_scr = smp.tile([128, D], fp32, tag="sqscr")
                for kt in range(KT):
                    nc.scalar.activation(
                        sq_scr, f_sb[:, kt], Act.Square,
                        accum_out=sal2[:, kt:kt + 1],
                    )
                salb = smp.tile([128, KT], fp32, tag="salb")
                nc.scalar.activation(salb, sal2, Act.Sqrt)
                invb = smp.tile([128, KT], fp32, tag="invb")
                nc.vector.reciprocal(invb, salb)

                # saliency into loop layout [1, N] via PE transpose
                salT_ps = ppB.tile([KT, 128], fp32, tag="tps")
                nc.tensor.transpose(salT_ps, salb, ident)
                salT = smp.tile([KT, 128], fp32, tag="salT")
                nc.scalar.copy(salT, salT_ps)
                nc.sync.dma_start(sal_loop[b:b + 1, :], salT[:, :])

                # scale f in place -> fn (unit rows)
                for kt in range(KT):
                    nc.vector.tensor_scalar(
                        f_sb[:, kt], f_sb[:, kt], invb[:, kt:kt + 1], None,
                        op0=Alu.mult,
                    )

                # transpose fn -> fnT [128(d), DT, N]
                fnT = ftp.tile([128, DT, N], fp32, tag="fnT")
                for kt in range(KT):
                    for dt in range(DT):
                        tp = ppB.tile([128, 128], fp32, tag="tps")
                        nc.tensor.transpose(
                            tp, f_sb[:, kt, dt * 128:(dt + 1) * 128], ident
                        )
                        if (kt + dt) % 2 == 0:
                            nc.scalar.copy(
                                fnT[:, dt, kt * 128:(kt + 1) * 128], tp
                            )
                        else:
                            nc.vector.tensor_copy(
                                fnT[:, dt, kt * 128:(kt + 1) * 128], tp
                            )

                # G = fnT.T @ fnT  (normalized Gram), row tiles -> DRAM
                for i in range(KT):
                    gps = ppA.tile([128, N], fp32, tag="gps")
                    for h in range(2):
                        for dt in range(DT):
                            nc.tensor.matmul(
                                gps[:, h * 512:(h + 1) * 512],
                                fnT[:, dt, i * 128:(i + 1) * 128],
                                fnT[:, dt, h * 512:(h + 1) * 512],
                                start=(dt == 0),
                                stop=(dt == DT - 1),
                            )
                    gstage = gsp.tile([128, N], fp32, tag="gstage")
                    nc.vector.tensor_copy(gstage[:, :512], gps[:, :512])
                    nc.scalar.copy(gstage[:, 512:], gps[:, 512:])
                    nc.sync.dma_start(
                        g_dr[b * N + i * 128: b * N + (i + 1) * 128, :], gstage
                    )

            # make sure all Gram writes to DRAM are visible before gathers
            tc.strict_bb_all_engine_barrier()

            # ---------------- Phase B: 16-step greedy loop -----------------
            mask = mp.tile([BPC, N], fp32)
            nc.vector.memset(mask, 1.0)
            msal = mp.tile([BPC, N], fp32)
            sim = mp.tile([BPC, N], fp32)
            mx8 = mp.tile([BPC, 8], fp32)
            idx8 = mp.tile([BPC, 8], u32)
            idxf = mp.tile([BPC, 1], fp32)
            rowidx = mp.tile([BPC, 1], i32)
            w1 = mp.tile([BPC, N], fp32)
            gate = mp.tile([BPC, N], fp32)
            aggw = mp.tile([BPC, N], fp32)
            aggw_bf = mp.tile([BPC, N], bf16)
            clipv = mp.tile([BPC, N], fp32)

            sim2 = mp.tile([BPC, N], fp32)
            w1b = mp.tile([BPC, N], fp32)
            sims = [sim, sim2]
            w1s = [w1, w1b]

            def emit_deferred(t):
                # off-critical aggregation work for step t (fills gather wait)
                s = sims[t % 2]
                w = w1s[t % 2]
                nc.vector.tensor_scalar(
                    gate, s, 0.5, None, op0=Alu.is_gt
                )
                nc.vector.tensor_mul(aggw, w, gate)
                nc.scalar.activation(
                    aggw_bf, aggw, Act.Copy,
                    accum_out=wsum[:, t:t + 1],
                )
                for kt in range(KT):
                    tp2 = ppB.tile([128, 128], fp32, tag="tps")
                    nc.tensor.transpose(
                        tp2[:, :BPC],
                        aggw[:, kt * 128:(kt + 1) * 128],
                        ident[:BPC, :BPC],
                    )
                    nc.scalar.copy(wT[:, kt, :, t], tp2[:, :BPC])

            for t in range(NS):
                s = sims[t % 2]
                nc.vector.tensor_mul(msal, sal_loop, mask)
                nc.vector.max(out=mx8, in_=msal)
                nc.vector.max_index(out=idx8, in_max=mx8, in_values=msal)
                nc.vector.tensor_copy(idxf, idx8[:, 0:1])
                nc.vector.tensor_scalar(
                    rowidx, idxf, rowb, None, op0=Alu.add
                )
                nc.gpsimd.indirect_dma_start(
                    out=s,
                    out_offset=None,
                    in_=g_dr,
                    in_offset=IndirectOffsetOnAxis(ap=rowidx, axis=0),
                )
                if t > 0:
                    emit_deferred(t - 1)
                # critical tail: uses gathered sim
                nc.vector.tensor_mul(w1s[t % 2], s, mask)
                nc.vector.tensor_scalar(
                    clipv, s, 0.0, 1.0, op0=Alu.max, op1=Alu.min
                )
                nc.vector.tensor_scalar(
                    clipv, clipv, -1.0, 1.0, op0=Alu.mult, op1=Alu.add
                )
                nc.vector.tensor_mul(mask, mask, clipv)
            emit_deferred(NS - 1)

            # ---------------- Phase C: slot matmuls ------------------------
            nc.vector.tensor_scalar(wsum, wsum, 1e-8, None, op0=Alu.add)
            recip = mp.tile([BPC, NS], fp32)
            nc.vector.reciprocal(recip, wsum)
            rT_ps = ppB.tile([128, 128], fp32, tag="tps")
            nc.tensor.transpose(rT_ps[:NS, :BPC], recip, ident[:BPC, :BPC])
            recipT = mp.tile([NS, BPC], fp32)
            # fold the quantization scale s1 back in: slot = sp * recip * s1
            nc.vector.tensor_scalar(
                recipT, rT_ps[:NS, :BPC], scl[:NS, 0:1], None, op0=Alu.mult
            )

            for b in range(BPC):
                f_c = load_f(b)
                sp = ppA.tile([NS, D], fp32, tag="gps")
                for h, (h0, h1) in enumerate([(0, 512), (512, D)]):
                    for kt in range(KT):
                        nc.tensor.matmul(
                            sp[:, h0:h1],
                            wT[:, kt, b, :],
                            f_c[:, kt, h0:h1],
                            start=(kt == 0),
                            stop=(kt == KT - 1),
                        )
                slot_sb = gsp.tile([NS, D], fp16, tag="slot")
                nc.scalar.activation(
                    slot_sb, sp, Act.Copy, scale=recipT[:, b:b + 1]
                )
                nc.sync.dma_start(out_dr[b], slot_sb)

    nc.compile()
    return nc


def _get_nc(debug=False):
    key = ("nc", debug)
    if key not in _CACHE:
        _CACHE[key] = _build_nc(debug)
    return _CACHE[key]


def _enable_jax_compile_cache():
    # run_bass_via_pjrt builds a fresh jax.jit closure per call; the
    # persistent cache turns each call's XLA recompile into a disk hit.
    if "jaxcache" in _CACHE:
        return
    _CACHE["jaxcache"] = True
    try:
        import tempfile
        import jax
        # fresh dir per process: warm calls in this process hit the cache
        # (run_bass_via_pjrt re-jits per call), but no process ever reloads
        # another's entries (cross-machine-feature AOT reload is flaky)
        cache_dir = tempfile.mkdtemp(prefix="kernel_jax_cache_")
        jax.config.update("jax_compilation_cache_dir", cache_dir)
        jax.config.update("jax_persistent_cache_min_compile_time_secs", 0.0)
        jax.config.update("jax_persistent_cache_min_entry_size_bytes", 0)
    except Exception:
        pass


def _quantize(features):
    """Host-side 20-bit encoding (int16 + packed int4) via jax-cpu jit."""
    import jax
    import jax.numpy as jnp

    cpu = jax.devices("cpu")[0]

    if "quant" not in _CACHE:
        def _q(f):
            # decimated max + 1.10 headroom; the clips below keep any
            # tail elements sound even if the subsample misses the true max
            decmax = jnp.max(jnp.abs(f[:, :, ::8]))
            s1 = decmax * np.float32(1.10) / np.float32(32767.0)
            inv_s1 = np.float32(1.0) / s1
            x = f * inv_s1
            q1f = jnp.clip(jnp.rint(x), -32767.0, 32767.0)
            q1 = q1f.astype(jnp.int16)
            r14 = jnp.clip((x - q1f) * 14.0, -7.0, 7.0)
            q4 = (jnp.rint(r14) + 8.0).astype(jnp.uint8)
            packed = q4[..., :DH] + q4[..., DH:] * jnp.uint8(16)
            return q1, packed, s1
        _CACHE["quant"] = jax.jit(_q, backend="cpu")

    f_np = np.ascontiguousarray(features, np.float32)
    try:
        with jax.default_device(cpu):
            f = jnp.from_dlpack(f_np)       # zero-copy np -> jax cpu
        assert f.dtype == jnp.float32
    except Exception:
        f = jax.device_put(f_np, cpu)
    q1, p4, s1 = _CACHE["quant"](f)
    try:
        q1 = np.from_dlpack(q1)
        p4 = np.from_dlpack(p4)
    except Exception:
        q1 = np.asarray(q1)
        p4 = np.asarray(p4)
    return q1, p4, np.float32(s1)


def kernel(features, batch_size=None, **_kw):
    from concourse import bass_utils

    _enable_jax_compile_cache()
    nc = _get_nc()
    q1, p4, s1 = _quantize(features)
    aux = np.zeros((128, 130), dtype=np.float32)
    aux[:, 0] = s1
    aux[:BPC, 1] = np.arange(BPC, dtype=np.float32) * N
    aux[:, 2:130] = np.eye(128, dtype=np.float32)
    in_maps = [
        {"q1": q1[i * BPC:(i + 1) * BPC], "p4": p4[i * BPC:(i + 1) * BPC],
         "aux": aux}
        for i in range(NC_CORES)
    ]
    res = bass_utils.run_bass_kernel_spmd(
        nc, in_maps, core_ids=list(range(NC_CORES))
    )
    outs = [np.asarray(res.results[i]["slots"]) for i in range(NC_CORES)]
    return np.concatenate(outs, axis=0).astype(np.float32)
